# revision 38
# baseline (speedup 1.0000x reference)
"""Trainium2 Bass kernel for the BinaryLayer problem.

Math: out[b,o] = OR_r ( mask[o,r] AND AND_t x_in[b, w[o,r,t]] ) with
x_in = [1 | x | 1-x].  AND over 16 literals == (sum of literal values == 16).
sum_t lit = base[j] + sum_f C[f,j]*x[b,f]  where for j=(o,r):
  C[f,j]  = (#slots with w==f+1) - (#slots with w==f+1+F)
  base[j] = (#slots with w==0) + (#slots with w>F)
Fold threshold+mask into a constant row: c1[j] = base[j]-16 (valid term) or
-1 (padded term, all w==0).  With S[b,j] = x_aug[b,:]@A[:,j] (A = [C; c1],
x_aug = [x, 1]):  AND true <=> S==0, and since S<=0 always,
out[b,o] = (max_r S[b,j(o,r)] == 0).  Exact small-int arithmetic in fp8e4m3
inputs / f32 PSUM accumulation; batch tiles 0,1,4,5 ship max_r S as f32 and
the host compares against -0.5; tiles 2,3,6,7 ship 32-wide fp8 residues
(sign-safe: every integer S <= -1 stays below -0.5 under fp8 rounding) and
the host finishes the tiny max.

Sharding (8 cores): 2-way over output columns x 4-way over batch.  Each core
gets its x^T batch columns and A column half as ONE host-packed fp8 buffer
xa = [A_b0 | x_g0 | A_b1 | x_g1 | A_b2 | A_b3] of [785, 3072] with
k = s*128 + p on device (8 subtiles; the 17-row tail subtile lands via one
small DMA and the padding zones are zeroed by a cheap u32-view memset so
every k-chunk runs as a uniform fp8 DoubleRow pair).  The column order puts
round 0's A bank adjacent to its x half so the first rounds stream as
row-sliced DMAs with minimal latency.

Device schedule: 8 rounds of (j-bank, 4-batch-tile group); each round is 16
DoubleRow matmuls (4 PSUM banks x 4 k-chunks of 256 rows, 107ns per 512-col
matmul at full p-state).  Per-round drains: DVE tensor_reduce takes the L
pair (2 banks) into y_f (f32); ACT copies the R pair to fp8 and the 32-wide
residue ships via the Pool/SWDGE path (off the shared HWDGE ring that
carries the input stream).  Rounds 4,5 fill bank-major (R banks first) so
their ACT copies free psR buffers early; rounds 6,7 run on four independent
1-bank PSUM tiles each (reusing the psL/psR tag rings) forming a drain
ladder, so after the very last matmul only one short 1-bank DVE reduce, two
small SP ships, one DMA-completion semaphore, and the queue-drain barrier
remain.  DoubleRow dummy matmuls on zeroed scratch warm the PE clock during
the initial DMA fill.
"""

import os

os.environ.setdefault("MYCRO_LOCAL_CACHE", "1")

import numpy as np

import concourse.bass as bass
import concourse.bacc as bacc
import concourse.mybir as mybir
from concourse.tile import TileContext
from concourse.bass_utils import run_bass_kernel_spmd

B, F = 4096, 784
OUT, OR_T, AND_T = 128, 32, 16
N_CORES = 8
JSH, BSH = 2, 4              # shard grid: 2-way over j columns, 4-way over batch
BS = B // BSH                # 1024 batch rows per core
JC = (OUT * OR_T) // JSH     # 2048 (o,r) columns per core
OC = OUT // JSH              # 64 output features per core
K = F + 1                    # 785 contraction rows (784 features + const)
KT = K - 768                 # 17 rows in the tail k-subtile
J = OUT * OR_T               # 4096 total (o,r) columns, j = o*32 + r
NBT = BS // 128              # 8 batch tiles per core
NBK = JC // 512              # 4 PSUM banks per batch tile
AW = 3072                    # total xa width
NWU = 16                     # PE warm-up matmuls (DoubleRow, 107-213ns each)
FP8 = mybir.dt.float8e4
FP8_NP = mybir.dt.np(FP8)
BF16 = mybir.dt.bfloat16

# Round order (j-bank, batch-tile group): banks interleave with groups so
# the A-bank DMA stream stays ahead of the tensor engine.
ROUNDS = [(0, 0), (1, 0), (0, 1), (2, 0), (1, 1), (3, 0), (2, 1), (3, 1)]

# Input DMA stream: (row0, row1, col0, col1) slices of xa, or "kt" for the
# 17-row k-tail block.  Order matters: SP/HWDGE processes one descriptor
# set per ~625ns and transfers serialize, so slices are ordered to land
# just before the round that consumes them.
STREAM = [
    (0, 256, 0, 1024), (256, 512, 0, 1024), (512, 768, 0, 1024), "kt",
    (0, 512, 1024, 1536), (512, 768, 1024, 1536),
    (0, 512, 1536, 2048), (512, 768, 1536, 2048),
    (0, 512, 2048, 2560), (512, 768, 2048, 2560),
    (0, 512, 2560, 3072), (512, 768, 2560, 3072),
]

_CACHE: dict = {}


def _build_nc(use_double_row: bool = True, ext: tuple | None = None) -> bass.Bass:
    assert use_double_row, "only the DoubleRow variant is implemented"
    if ext is None:
        ext = _CACHE.get("ext", (32,) * NBK)
    f32 = mybir.dt.float32
    mmax = mybir.AluOpType.max
    X = mybir.AxisListType.X
    nc = bacc.Bacc("TRN2")
    xa_d = nc.declare_dram_parameter("xa", [K, AW], FP8, isOutput=False)
    y_d = nc.declare_dram_parameter("y", [128, NBT, OC], f32, isOutput=True)
    yR_d = nc.declare_dram_parameter("yR", [128, 4, NBK, 16, 32], FP8, isOutput=True)

    with TileContext(nc) as tc:
        with (
            tc.tile_pool(name="const", bufs=1) as cpool,
            tc.tile_pool(name="psum", bufs=2, space="PSUM") as ppool,
            tc.tile_pool(name="work", bufs=2) as wpool,
        ):
            # xa in SBUF as [partition p, subtile s, col] with k = s*128 + p.
            xa_sb = cpool.tile([128, 8, AW], FP8)
            wu = cpool.tile([128, 2, 640], FP8)
            y_f = cpool.tile([128, NBT, NBK, 16], f32)

            # t=0 work on the otherwise-idle gpsimd: zero the warm-up
            # scratch and the k-padding zones (u32 views are 4x cheaper);
            # wu first so warm-ups start immediately after.
            u32 = mybir.dt.uint32
            nc.gpsimd.memset(wu[:].bitcast(u32), 0)
            nc.gpsimd.memset(xa_sb[:, 6:8, :].bitcast(u32), 0)

            # Input DMAs on SP/HWDGE, sliced/ordered to match consumption.
            # Host packs xa as [A_b0 | x_g0 | A_b1 | x_g1 | A_b2 | A_b3] so
            # round 0's A bank and x half stream as ONE row-sliced sequence;
            # later regions are row-sliced so each round's first-needed
            # subtile pair sems ~1µs before the rest.
            def load(eng, rows, cols):
                eng.dma_start(
                    out=xa_sb[:, rows.start // 128 : rows.stop // 128, cols],
                    in_=xa_d[rows, cols].rearrange("(s p) j -> p s j", p=128),
                )

            for item in STREAM:
                if item == "kt":
                    nc.sync.dma_start(
                        out=xa_sb[0:KT, 6, :], in_=xa_d[768:K, :]
                    )
                else:
                    r0, r1, c0, c1_ = item
                    load(nc.sync, slice(r0, r1), slice(c0, c1_))

            ACOL = (0, 1024, 2048, 2560)  # A bank jq start column in xa
            XCOL = (512, 1536)            # x half start column per group

            def matmul(ps_out, sp, bt, jq, start, stop):
                # sp 0..2: full 512-col chunks (start on 0, stop on 2);
                # sp 3: the 17-row tail chunk streams only each o's touched
                # prefix (extent per j-bank), start=False/stop=True.
                ssl = slice(2 * sp, 2 * sp + 2)
                asl = slice(ACOL[jq], ACOL[jq] + 512)
                xc = XCOL[bt // 4] + (bt % 4) * 128
                e = ext[jq]
                if sp == 3 and e < 32:
                    nc.tensor.matmul(
                        ps_out[:, :, 0:e],
                        xa_sb[:, ssl, xc : xc + 128],
                        xa_sb[:, ssl, asl].rearrange(
                            "p s (o r) -> p s o r", r=OR_T
                        )[:, :, :, 0:e],
                        start=False,
                        stop=True,
                        perf_mode=mybir.MatmulPerfMode.DoubleRow,
                    )
                else:
                    nc.tensor.matmul(
                        ps_out,
                        xa_sb[:, ssl, xc : xc + 128],
                        xa_sb[:, ssl, asl],
                        start=start,
                        stop=(stop if e >= 32 else sp == 2),
                        perf_mode=mybir.MatmulPerfMode.DoubleRow,
                    )

            # Per-round drains.  Hardware limits: only DVE tensor_reduce
            # and ACT activation may read PSUM (one PSUM operand), and
            # gpsimd has no ALU ops on TRN2 — so each round's 4 banks are
            # TWO independent 2-bank tiles: DVE direct-reduces tile L into
            # y_f while ACT copies tile R to fp8; the 32-wide fp8 residue
            # ships via Pool/SWDGE and the host finishes the tiny max.
            # DVE (1192ns) and ACT (1038ns) stay under the ~1424ns round
            # period, so the tensor engine paces the kernel.
            Copy = mybir.ActivationFunctionType.Copy
            for r, (jq, g) in enumerate(ROUNDS[:-2]):
                gs = 4 * g
                psL = ppool.tile([128, 2, 16, 32], f32, name="psL", tag="psL")
                psR = ppool.tile([128, 2, 16, 32], f32, name="psR", tag="psR")
                if r == 0:
                    for _ in range(NWU):
                        nc.tensor.matmul(
                            psL[:, 0], wu[:, :, 0:128], wu[:, :, 128:640],
                            start=True, stop=True,
                            perf_mode=mybir.MatmulPerfMode.DoubleRow,
                        )
                if r >= 4:
                    # Late rounds fill bank-major, R banks first: the ACT
                    # copy then starts mid-round, freeing the psR buffer
                    # ~700ns earlier for the 1-bank ladder of rounds 6/7.
                    # (Early rounds stay sp-major so each 256-row DMA slice
                    # feeds all four banks as it lands.)
                    for i in (2, 3, 0, 1):
                        for sp in range(4):
                            matmul(
                                psL[:, i] if i < 2 else psR[:, i - 2],
                                sp, gs + i, jq, sp == 0, sp == 3,
                            )
                else:
                    for sp in range(4):
                        for i in range(4):
                            matmul(
                                psL[:, i] if i < 2 else psR[:, i - 2],
                                sp, gs + i, jq, sp == 0, sp == 3,
                            )
                nc.vector.tensor_reduce(
                    out=y_f[:, gs : gs + 2, jq, :], in_=psL[:], axis=X, op=mmax
                )
                c1 = wpool.tile(
                    [128, 2, 16, 32], FP8, name="c1", tag="c1", bufs=4
                )
                nc.scalar.activation(out=c1[:], in_=psR[:], func=Copy)
                nc.gpsimd.dma_start(
                    out=yR_d[:, 2 * g : 2 * g + 2, jq], in_=c1[:]
                )
                if ROUNDS[r][1] == 0 and all(gg == 1 for _, gg in ROUNDS[r + 1 :]):
                    nc.sync.dma_start(out=y_d[:, 0:2, :], in_=y_f[:, 0:2])
                if r == max(ROUNDS.index((0, 1)), ROUNDS.index((1, 1))):
                    # banks 0,1 of the g1 L-pair are final once both their
                    # rounds have run; ship them mid-stream so only banks
                    # 2,3 ride the tail.
                    nc.gpsimd.dma_start(
                        out=y_d[:, 4:6, 0:32], in_=y_f[:, 4:6, 0:2]
                    )

            # Rounds 6 and 7 run on 1-bank PSUM tiles (the psL/psR tag rings
            # provide two independent buffers each) so every bank drains the
            # moment it stops accumulating — a 2-bank tile would serialize a
            # region drain against fills of its other bank.  Round 6 fills
            # bank-major [R0, L0, R1, L1]: its R banks leave via two 1-bank
            # ACT fp8 copies + one Pool yR ship, its L banks via two 1-bank
            # DVE reduces, and each buffer frees just in time for round 7's
            # ladder.
            jq, g = ROUNDS[-2]
            r6R0 = ppool.tile([128, 16, 32], f32, name="r6R0", tag="psR")
            r6L0 = ppool.tile([128, 16, 32], f32, name="r6L0", tag="psL")
            r6R1 = ppool.tile([128, 16, 32], f32, name="r6R1", tag="psR")
            r6L1 = ppool.tile([128, 16, 32], f32, name="r6L1", tag="psL")
            c6 = wpool.tile(
                [128, 2, 16, 32], FP8, name="c6", tag="c1", bufs=4
            )
            for sp in range(4):
                matmul(r6R0, sp, 4 * g + 2, jq, sp == 0, sp == 3)
            nc.scalar.activation(out=c6[:, 0], in_=r6R0[:], func=Copy)
            for sp in range(4):
                matmul(r6L0, sp, 4 * g + 0, jq, sp == 0, sp == 3)
            nc.vector.tensor_reduce(
                out=y_f[:, 4 * g, jq, :], in_=r6L0[:], axis=X, op=mmax
            )
            for sp in range(4):
                matmul(r6R1, sp, 4 * g + 3, jq, sp == 0, sp == 3)
            nc.scalar.activation(out=c6[:, 1], in_=r6R1[:], func=Copy)
            nc.gpsimd.dma_start(out=yR_d[:, 2 * g : 2 * g + 2, jq], in_=c6[:])
            for sp in range(4):
                matmul(r6L1, sp, 4 * g + 1, jq, sp == 0, sp == 3)
            nc.vector.tensor_reduce(
                out=y_f[:, 4 * g + 1, jq, :], in_=r6L1[:], axis=X, op=mmax
            )
            nc.sync.dma_start(
                out=y_d[:, 4:6, 32:48], in_=y_f[:, 4:6, 2]
            )

            # Last round, drain ladder [bt6, bt4, bt7, bt5]: the ACT pair
            # (bt6/7 -> fp8 32-wide yR residues in one shared tile, shipped
            # as ONE SP DMA after the second copy) and the DVE pair (bt4/5
            # -> 1-bank reduces into y_f) interleave so each drain starts
            # the moment its bank stops accumulating.  The final SP ship
            # carries only the 32 bank-2/3 columns of y_f[:, 4:6] (banks
            # 0,1 shipped after round 5).
            jq, g = ROUNDS[-1]
            pr0 = ppool.tile([128, 16, 32], f32, name="pr0", tag="psR")
            pl0 = ppool.tile([128, 16, 32], f32, name="pl0", tag="psL")
            pr1 = ppool.tile([128, 16, 32], f32, name="pr1", tag="psR")
            pl1 = ppool.tile([128, 16, 32], f32, name="pl1", tag="psL")
            cl = wpool.tile([128, 2, 16, 32], FP8, name="cl", tag="cl")
            for sp in range(4):
                matmul(pr0, sp, 4 * g + 2, jq, sp == 0, sp == 3)
            nc.scalar.activation(out=cl[:, 0], in_=pr0[:], func=Copy)
            for sp in range(4):
                matmul(pl0, sp, 4 * g + 0, jq, sp == 0, sp == 3)
            nc.vector.tensor_reduce(
                out=y_f[:, 4, jq, :], in_=pl0[:], axis=X, op=mmax
            )
            for sp in range(4):
                matmul(pr1, sp, 4 * g + 3, jq, sp == 0, sp == 3)
            nc.scalar.activation(out=cl[:, 1], in_=pr1[:], func=Copy)
            nc.scalar.dma_start(out=yR_d[:, 2:4, jq], in_=cl[:])
            for sp in range(4):
                matmul(pl1, sp, 4 * g + 1, jq, sp == 0, sp == 3)
            nc.vector.tensor_reduce(
                out=y_f[:, 5, jq, :], in_=pl1[:], axis=X, op=mmax
            )
            nc.sync.dma_start(out=y_d[:, 4:6, 48:64], in_=y_f[:, 4:6, 3])
    return nc


def _get_nc() -> bass.Bass:
    ext = _CACHE.get("ext", (32,) * NBK)
    key = ("nc", ext)
    if key not in _CACHE:
        nc = _build_nc(use_double_row=True, ext=ext)
        nc.finalize()
        _CACHE[key] = nc
    return _CACHE[key]


def _tail_features(weights: np.ndarray) -> np.ndarray:
    """Greedily pick the 17 features of the tail k-chunk to minimize the
    summed per-j-bank extents (max per-o count of terms touching the set)."""
    w = weights.reshape(J, AND_T).astype(np.int64)
    M = np.zeros((F, J), bool)  # feature -> terms using it
    for f in range(F):
        M[f] = ((w == f + 1) | (w == f + 1 + F)).any(-1)
    touched = np.zeros(J, bool)
    chosen = []
    for _ in range(KT):
        t2 = touched[None, :] | M  # [F, J]
        cnt = t2.reshape(F, OUT, OR_T).sum(-1)  # [F, OUT]
        # o's are later re-assigned to banks by descending count (16 per
        # bank), so the cost of a candidate set is the sum over both core
        # halves of the sorted-descending quartile heads.
        s0 = np.sort(cnt[:, :OC], 1)[:, ::-1]
        s1 = np.sort(cnt[:, OC:], 1)[:, ::-1]
        q = np.arange(0, OC, 16)
        score = (
            np.maximum(s0[:, q], s1[:, q]).sum(1) + 1e-6 * t2.sum(1)
        )
        score[chosen] = np.inf
        f = int(np.argmin(score))
        chosen.append(f)
        touched |= M[f]
    return np.array(chosen)


def _build_A(weights: np.ndarray):
    """[K, J] fp8 literal-count matrix (row 0 the folded threshold/mask
    constant) with a greedily-chosen 17-feature tail chunk (rows 768..784)
    and each o's 32 r-columns permuted so terms touching the tail come
    first, plus the per-j-bank extents and the feature row permutation.
    The device streams only the touched prefix in the tail k-chunk; max_r
    is permutation invariant so outputs need no fixup."""
    tailf = _tail_features(weights)
    perm = np.concatenate([np.setdiff1d(np.arange(F), tailf), tailf])
    rowof = np.empty(F, np.int64)
    rowof[perm] = 1 + np.arange(F)  # feature f lives in row rowof[f]
    w = weights.reshape(J, AND_T).astype(np.int64)
    v = w.reshape(-1)
    j_idx = np.repeat(np.arange(J), AND_T)
    C = np.zeros((K, J), np.float32)
    pos = (v >= 1) & (v <= F)
    neg = v > F
    np.add.at(C, (rowof[v[pos] - 1], j_idx[pos]), 1.0)
    np.add.at(C, (rowof[v[neg] - 1 - F], j_idx[neg]), -1.0)
    base = (w == 0).sum(1) + neg.reshape(J, AND_T).sum(1)
    padded = (w == 0).all(1)
    C[0, :] = np.where(padded, -1.0, base - 16.0).astype(np.float32)
    A8 = C.astype(FP8_NP)
    assert np.array_equal(A8.astype(np.float32), C), "fp8 must be exact"
    wor = weights.astype(np.int64)  # [OUT, OR_T, AND_T]
    tl = np.isin(wor - 1, tailf) & (wor >= 1) & (wor <= F)
    tl |= np.isin(wor - 1 - F, tailf) & (wor > F)
    touched = tl.any(-1)  # [OUT, OR_T]
    order = np.argsort(~touched, axis=1, kind="stable")  # touched r's first
    cnt = touched.sum(1)
    A8 = A8.reshape(K, OUT, OR_T)[:, np.arange(OUT)[:, None], order]
    # Assign o's to banks by descending tail-count (16 per bank, per core
    # half): bank maxes then shrink from the raw layout's ~(10,11,10,11) to
    # ~(11,9,9,7), cutting every tail matmul's streamed extent.  The o
    # permutation is undone on the host.
    operm = np.stack(
        [
            np.argsort(-cnt[h * OC : (h + 1) * OC], kind="stable")
            for h in range(JSH)
        ]
    )
    A8p = np.empty_like(A8)
    cntp = np.empty_like(cnt)
    for h in range(JSH):
        A8p[:, h * OC : (h + 1) * OC] = A8[:, h * OC + operm[h]]
        cntp[h * OC : (h + 1) * OC] = cnt[h * OC + operm[h]]
    A8 = A8p.reshape(K, J)
    ext = []
    for jq in range(NBK):
        ext.append(
            int(
                max(
                    cntp[h * OC + jq * 16 : h * OC + (jq + 1) * 16].max()
                    for h in range(JSH)
                )
            )
        )
    return A8, tuple(ext), perm, operm


def make_in_maps(x: np.ndarray, weights: np.ndarray) -> list[dict]:
    A8, ext, perm, operm = _build_A(weights)
    _CACHE["ext"] = ext
    _CACHE["operm"] = operm
    xT = np.empty((K, B), FP8_NP)
    xT[0, :] = 1.0
    xT[1:, :] = x.T[perm].astype(FP8_NP)
    maps = []
    for c in range(N_CORES):
        jb, bs = c // BSH, c % BSH
        Ac = A8[:, jb * JC : (jb + 1) * JC]
        xc = xT[:, bs * BS : (bs + 1) * BS]
        # Column layout [A_b0 | x_g0 | A_b1 | x_g1 | A_b2 | A_b3] keeps
        # round 0's A bank adjacent to its x half (one row-sliced DMA
        # stream) — must match ACOL/XCOL in _build_nc.
        xa = np.concatenate(
            [
                Ac[:, 0:512], xc[:, 0:512],
                Ac[:, 512:1024], xc[:, 512:1024],
                Ac[:, 1024:1536], Ac[:, 1536:2048],
            ],
            axis=1,
        )
        maps.append({"xa": np.ascontiguousarray(xa)})
    return maps


def kernel(x: np.ndarray, weights: np.ndarray) -> np.ndarray:
    x = np.asarray(x)
    weights = np.asarray(weights)
    in_maps = make_in_maps(x, weights)
    nc = _get_nc()
    res = run_bass_kernel_spmd(nc, in_maps, list(range(N_CORES)))
    out = np.empty((B, OUT), dtype=bool)
    for c in range(N_CORES):
        jb, bs = c // BSH, c % BSH
        # Batch tiles 0,1,4,5 arrive fully reduced in y (max_r S, ints <= 0;
        # True <=> S == 0).  Tiles 2,3,6,7 arrive as 32-wide fp8 residues in
        # yR [p, slot, jq, o, r32]; the host finishes the max (fp8 rounding
        # keeps every negative integer <= -1 below -0.5, so the threshold
        # test is exact).
        y = res.results[c]["y"]          # [128, NBT, OC] f32
        yR = res.results[c]["yR"].astype(np.float32)  # [128, 4, NBK, 16, 32]
        # device column position p holds real output o = operm[jb][p]
        csl = jb * OC + _CACHE["operm"][jb]

        def rows(bt):
            lo = bs * BS + bt * 128
            return slice(lo, lo + 128)

        for g in range(2):
            for i in range(2):
                out[rows(4 * g + i), csl] = y[:, 4 * g + i, :] >= -0.5
                r = yR[:, 2 * g + i].max(-1)  # [128, NBK, 16]
                out[rows(4 * g + 2 + i), csl] = r.reshape(128, OC) >= -0.5
    return out
